# revision 26
# baseline (speedup 1.0000x reference)
"""Custom cross-entropy-with-top-k loss kernel for Trainium2 (8 NeuronCores).

Reference computation (B=16384 rows, C=8192 classes, K=5, POWER=1.01):
    log_prob      = log_softmax(input)
    topk_vals     = top-5 values per row
    log_prob_topk = log(1.01^topk_vals / sum(1.01^topk_vals))
    log_prob_copy = log_prob with topk positions overwritten by log_prob_topk
    loss = mean(-log_prob[r, target[r]]) + mean(-log_prob_copy[r, target[r]])

Key reduction: the scalar loss needs only, per row,
    lse   = log(sum(exp(x)))               (x ~ N(0,1): exp() safe in f32)
    x_t   = x[row, target[row]]            (indirect-DMA gather)
    top5  = 5 largest values               (VectorE InstMax = top-8)
    sel   = x_t >= top5[4]                 (is target among the top-5)
    lp2   = sel ? ln(1.01)*x_t - log(sum(1.01^top5)) : x_t - lse
    term  = (lse - x_t) - lp2
and the answer is mean(term).  Per core: 2048 rows = 16 tiles of 128
partitions x 8192 f32, streamed at the HBM roofline.  Per tile: one
4 MiB HWDGE load into a 4-buffer rotation, one ScalarE Exp pass with a
per-row accumulator, one VectorE top-8 pass.  The epilogue works on
[128, <=80] tiles.

Written in raw Bass (no Tile scheduler): the neuronxcc walrus backend
only encodes ONE semaphore wait per TPB instruction, so synchronization
uses explicit standalone wait_ge instructions (one wait each) and
relies on transitive ordering (e.g. a load's slot-WAW is implied by
waiting on the consumers of the previous load, which themselves waited
on that load's completion).
"""

import numpy as np

P = 128                    # SBUF partitions
C = 8192                   # classes
NTILES = 16                # row-tiles per core
B_LOCAL = P * NTILES       # 2048 rows per core
N_CORES = 8
B = B_LOCAL * N_CORES      # 16384
LN101 = float(np.log(np.float64(1.01)))

NB = 4                     # x-tile rotation depth
_CACHE = {}


def _build_bass():
    from contextlib import ExitStack

    import concourse.bass as bass
    import concourse.mybir as mybir

    nc = bass.Bass()
    f32 = mybir.dt.float32
    x = nc.declare_dram_parameter("x", [B_LOCAL, C], f32, isOutput=False)
    gidx = nc.declare_dram_parameter(
        "gidx", [P, NTILES], mybir.dt.int32, isOutput=False
    )
    out = nc.declare_dram_parameter("out", [P, 1], f32, isOutput=True)

    Exp = mybir.ActivationFunctionType.Exp
    Ln = mybir.ActivationFunctionType.Ln
    X = mybir.AxisListType.X
    Alu = mybir.AluOpType

    with ExitStack() as ctx:
        xt = [
            ctx.enter_context(nc.sbuf_tensor(f"xt{j}", [P, C], f32))
            for j in range(NB)
        ]
        exp_scr = [
            ctx.enter_context(nc.sbuf_tensor(f"exp_scr{j}", [P, C], f32))
            for j in range(2)
        ]
        gidx_sb = ctx.enter_context(
            nc.sbuf_tensor("gidx_sb", [P, NTILES], mybir.dt.int32)
        )
        xt_all = ctx.enter_context(nc.sbuf_tensor("xt_all", [P, NTILES], f32))
        top8_all = ctx.enter_context(
            nc.sbuf_tensor("top8_all", [P, NTILES, 8], f32)
        )
        sumexp_all = ctx.enter_context(
            nc.sbuf_tensor("sumexp_all", [P, NTILES], f32)
        )
        pw_all = ctx.enter_context(nc.sbuf_tensor("pw_all", [P, NTILES, 5], f32))
        lse_all = ctx.enter_context(nc.sbuf_tensor("lse_all", [P, NTILES], f32))
        s_red = ctx.enter_context(nc.sbuf_tensor("s_red", [P, NTILES], f32))
        logs_all = ctx.enter_context(
            nc.sbuf_tensor("logs_all", [P, NTILES], f32)
        )
        a_all = ctx.enter_context(nc.sbuf_tensor("a_all", [P, NTILES], f32))
        d_all = ctx.enter_context(nc.sbuf_tensor("d_all", [P, NTILES], f32))
        sel_all = ctx.enter_context(nc.sbuf_tensor("sel_all", [P, NTILES], f32))
        term_all = ctx.enter_context(
            nc.sbuf_tensor("term_all", [P, NTILES], f32)
        )
        partial = ctx.enter_context(nc.sbuf_tensor("partial", [P, 1], f32))

        s_gidx = ctx.enter_context(nc.semaphore("s_gidx"))
        # One semaphore per tile load: a semaphore's first increment (0->16)
        # needs no prior wait by the enqueuing engine, so the DMA queue can
        # run arbitrarily far ahead without completion-order hazards.
        s_load = [
            ctx.enter_context(nc.semaphore(f"s_load{i}")) for i in range(NTILES)
        ]
        s_store = ctx.enter_context(nc.semaphore("s_store"))
        s_gather = ctx.enter_context(nc.semaphore("s_gather"))
        s_act = ctx.enter_context(nc.semaphore("s_act"))
        s_dve = ctx.enter_context(nc.semaphore("s_dve"))
        block = ctx.enter_context(nc.Block())

        @block.sync
        def _(sync):
            sync.dma_start(out=gidx_sb[:, :], in_=gidx[:, :]).then_inc(s_gidx, 16)
            for i in range(NTILES):
                if i >= NB:
                    # Slot reuse: wait for both compute consumers of the
                    # previous occupant.  Their completion also implies that
                    # load's completion (they waited on s_load), covering
                    # the slot WAW transitively.
                    sync.wait_ge(s_act, i - NB + 1)
                    sync.wait_ge(s_dve, i - NB + 1)
                sync.dma_start(
                    out=xt[i % NB][:, :], in_=x[i * P : (i + 1) * P, :]
                ).then_inc(s_load[i], 16)
            # final store after the whole epilogue
            sync.wait_ge(s_dve, NTILES + 8)
            sync.dma_start(out=out[:, :], in_=partial[:, :]).then_inc(s_store, 16)

        @block.gpsimd
        def _(gpsimd):
            gpsimd.wait_ge(s_gidx, 16)
            x_flat = bass.AP(tensor=x, offset=0, ap=[[1, B_LOCAL * C], [1, 1]])
            gpsimd.indirect_dma_start(
                out=xt_all[:, :],
                out_offset=None,
                in_=x_flat,
                in_offset=bass.IndirectOffsetOnAxis(ap=gidx_sb[:, :], axis=0),
            ).then_inc(s_gather, 16)

        @block.scalar
        def _(scalar):
            for i in range(NTILES):
                scalar.wait_ge(s_load[i], 16)
                if i >= 2:
                    # WAW on the double-buffered scratch: wait for the exp
                    # two tiles back (lagged, so the pipeline never bubbles).
                    scalar.wait_ge(s_act, i - 1)
                scalar.activation(
                    out=exp_scr[i % 2][:, :],
                    in_=xt[i % NB][:, :],
                    func=Exp,
                    accum_out=sumexp_all[:, i : i + 1],
                ).then_inc(s_act, 1)
            # epilogue: 1.01^v on the top-5, then the two logs
            scalar.wait_ge(s_dve, NTILES)
            scalar.activation(
                out=pw_all[:, :, :],
                in_=top8_all[:, :, 0:5],
                func=Exp,
                scale=LN101,
            ).then_inc(s_act, 1)  # -> NTILES+1
            # lse: reads this engine's own accumulator outputs; guard the
            # deep pipeline with a self-wait.
            scalar.wait_ge(s_act, NTILES)
            scalar.activation(
                out=lse_all[:, :], in_=sumexp_all[:, :], func=Ln
            ).then_inc(s_act, 1)  # -> NTILES+2
            scalar.wait_ge(s_dve, NTILES + 1)  # s_red ready
            scalar.activation(
                out=logs_all[:, :], in_=s_red[:, :], func=Ln
            ).then_inc(s_act, 1)  # -> NTILES+3

        @block.vector
        def _(vector):
            for i in range(NTILES):
                vector.wait_ge(s_load[i], 16)
                vector.max(out=top8_all[:, i, :], in_=xt[i % NB][:, :]).then_inc(
                    s_dve, 1
                )
            # epilogue
            vector.wait_ge(s_act, NTILES + 1)  # pw_all ready
            vector.reduce_sum(out=s_red[:, :], in_=pw_all[:, :, :], axis=X).then_inc(
                s_dve, 1
            )  # -> NTILES+1
            vector.wait_ge(s_gather, 16)
            vector.wait_ge(s_act, NTILES + 3)  # lse + logs ready
            # Each dependent step self-waits on the previous DVE increment:
            # the DVE pipeline gives no same-engine RAW ordering guarantee.
            # a = lse - x_t  (= -log_prob[target])
            vector.tensor_sub(
                out=a_all[:, :], in0=lse_all[:, :], in1=xt_all[:, :]
            ).then_inc(s_dve, 1)  # -> N+2
            # d = (logS - ln(1.01)*x_t) - a
            vector.scalar_tensor_tensor(
                out=d_all[:, :],
                in0=xt_all[:, :],
                scalar=-LN101,
                in1=logs_all[:, :],
                op0=Alu.mult,
                op1=Alu.add,
            ).then_inc(s_dve, 1)  # -> N+3
            vector.wait_ge(s_dve, NTILES + 3)
            vector.tensor_sub(
                out=d_all[:, :], in0=d_all[:, :], in1=a_all[:, :]
            ).then_inc(s_dve, 1)  # -> N+4
            # sel = x_t >= 5th-largest value
            vector.tensor_tensor(
                out=sel_all[:, :],
                in0=xt_all[:, :],
                in1=top8_all[:, :, 4],
                op=Alu.is_ge,
            ).then_inc(s_dve, 1)  # -> N+5
            vector.wait_ge(s_dve, NTILES + 5)
            vector.tensor_mul(
                out=d_all[:, :], in0=sel_all[:, :], in1=d_all[:, :]
            ).then_inc(s_dve, 1)  # -> N+6
            # term = 2*a + sel*d  (= (lse-x_t) - lp2)
            vector.wait_ge(s_dve, NTILES + 6)
            vector.scalar_tensor_tensor(
                out=term_all[:, :],
                in0=a_all[:, :],
                scalar=2.0,
                in1=d_all[:, :],
                op0=Alu.mult,
                op1=Alu.add,
            ).then_inc(s_dve, 1)  # -> N+7
            vector.wait_ge(s_dve, NTILES + 7)
            vector.reduce_sum(out=partial[:, :], in_=term_all[:, :], axis=X).then_inc(
                s_dve, 1
            )  # -> N+8

    return nc


def get_bass():
    if "nc" not in _CACHE:
        _CACHE["nc"] = _build_bass()
    return _CACHE["nc"]


def make_in_maps(input, target):
    """Shard the full inputs into per-core input maps."""
    x = np.ascontiguousarray(np.asarray(input, dtype=np.float32))
    t = np.asarray(target).astype(np.int64)
    assert x.shape == (B, C), x.shape
    assert t.shape == (B,), t.shape
    rows_local = np.arange(B_LOCAL, dtype=np.int64)
    in_maps = []
    for k in range(N_CORES):
        lo = k * B_LOCAL
        flat_idx = rows_local * C + t[lo : lo + B_LOCAL]
        # gidx[p, i] = flat offset of local row i*P + p
        gidx_k = np.ascontiguousarray(
            flat_idx.reshape(NTILES, P).T.astype(np.int32)
        )
        in_maps.append({"x": x[lo : lo + B_LOCAL], "gidx": gidx_k})
    return in_maps


def reduce_outputs(results):
    """Combine per-core [P, 1] partial sums into the scalar loss."""
    total = np.float64(0.0)
    for r in results:
        total += np.asarray(r["out"], dtype=np.float64).sum()
    return np.float32(total / B)


def kernel(input, target):
    from concourse.bass_utils import run_bass_kernel_spmd

    nc = get_bass()
    in_maps = make_in_maps(input, target)
    res = run_bass_kernel_spmd(nc, in_maps, list(range(N_CORES)))
    return reduce_outputs(res.results)


# revision 27
# speedup vs baseline: 24.9750x; 24.9750x over previous
"""Custom cross-entropy-with-top-k loss kernel for Trainium2 (8 NeuronCores).

Reference computation (B=16384 rows, C=8192 classes, K=5, POWER=1.01):
    log_prob      = log_softmax(input)
    topk_vals     = top-5 values per row
    log_prob_topk = log(1.01^topk_vals / sum(1.01^topk_vals))
    log_prob_copy = log_prob with topk positions overwritten by log_prob_topk
    loss = mean(-log_prob[r, target[r]]) + mean(-log_prob_copy[r, target[r]])

Key reduction: the scalar loss needs only, per row,
    lse   = log(sum(exp(x)))               (x ~ N(0,1): exp() safe in f32)
    x_t   = x[row, target[row]]            (indirect-DMA gather)
    top5  = 5 largest values               (VectorE InstMax = top-8)
    sel   = x_t >= top5[4]                 (is target among the top-5)
    lp2   = sel ? ln(1.01)*x_t - log(sum(1.01^top5)) : x_t - lse
    term  = (lse - x_t) - lp2
and the answer is mean(term).  Per core: 2048 rows = 16 tiles of 128
partitions x 8192 f32, streamed at the HBM roofline.  Per tile: one
4 MiB HWDGE load into a 4-buffer rotation, one ScalarE Exp pass with a
per-row accumulator, one VectorE top-8 pass.  The epilogue works on
[128, <=80] tiles.

Written in raw Bass (no Tile scheduler): the neuronxcc walrus backend
only encodes ONE semaphore wait per TPB instruction, so synchronization
uses explicit standalone wait_ge instructions (one wait each) and
relies on transitive ordering (e.g. a load's slot-WAW is implied by
waiting on the consumers of the previous load, which themselves waited
on that load's completion).
"""

import numpy as np

P = 128                    # SBUF partitions
C = 8192                   # classes
NTILES = 16                # row-tiles per core
B_LOCAL = P * NTILES       # 2048 rows per core
N_CORES = 8
B = B_LOCAL * N_CORES      # 16384
LN101 = float(np.log(np.float64(1.01)))

NB = 4                     # x-tile rotation depth
_CACHE = {}


def _build_bass(reps=1):
    from contextlib import ExitStack

    import concourse.bass as bass
    import concourse.mybir as mybir

    nc = bass.Bass()
    f32 = mybir.dt.float32
    x = nc.declare_dram_parameter("x", [B_LOCAL, C], f32, isOutput=False)
    gidx = nc.declare_dram_parameter(
        "gidx", [P, NTILES], mybir.dt.int32, isOutput=False
    )
    out = nc.declare_dram_parameter("out", [P, 1], f32, isOutput=True)

    Exp = mybir.ActivationFunctionType.Exp
    Ln = mybir.ActivationFunctionType.Ln
    X = mybir.AxisListType.X
    Alu = mybir.AluOpType

    with ExitStack() as ctx:
        xt = [
            ctx.enter_context(nc.sbuf_tensor(f"xt{j}", [P, C], f32))
            for j in range(NB)
        ]
        exp_scr = [
            ctx.enter_context(nc.sbuf_tensor(f"exp_scr{j}", [P, C], f32))
            for j in range(2)
        ]
        gidx_sb = ctx.enter_context(
            nc.sbuf_tensor("gidx_sb", [P, NTILES], mybir.dt.int32)
        )
        xt_all = ctx.enter_context(nc.sbuf_tensor("xt_all", [P, NTILES], f32))
        top8_all = ctx.enter_context(
            nc.sbuf_tensor("top8_all", [P, NTILES, 8], f32)
        )
        sumexp_all = ctx.enter_context(
            nc.sbuf_tensor("sumexp_all", [P, NTILES], f32)
        )
        pw_all = ctx.enter_context(nc.sbuf_tensor("pw_all", [P, NTILES, 5], f32))
        lse_all = ctx.enter_context(nc.sbuf_tensor("lse_all", [P, NTILES], f32))
        s_red = ctx.enter_context(nc.sbuf_tensor("s_red", [P, NTILES], f32))
        logs_all = ctx.enter_context(
            nc.sbuf_tensor("logs_all", [P, NTILES], f32)
        )
        a_all = ctx.enter_context(nc.sbuf_tensor("a_all", [P, NTILES], f32))
        d_all = ctx.enter_context(nc.sbuf_tensor("d_all", [P, NTILES], f32))
        sel_all = ctx.enter_context(nc.sbuf_tensor("sel_all", [P, NTILES], f32))
        term_all = ctx.enter_context(
            nc.sbuf_tensor("term_all", [P, NTILES], f32)
        )
        partial = ctx.enter_context(nc.sbuf_tensor("partial", [P, 1], f32))

        s_gidx = ctx.enter_context(nc.semaphore("s_gidx"))
        # One semaphore per tile load: a semaphore's first increment (0->16)
        # needs no prior wait by the enqueuing engine, so the DMA queue can
        # run arbitrarily far ahead without completion-order hazards.
        s_load = [
            ctx.enter_context(nc.semaphore(f"s_load{i}")) for i in range(NTILES)
        ]
        NT = NTILES * reps  # total tile passes (reps>1 only for benchmarking)
        s_store = ctx.enter_context(nc.semaphore("s_store"))
        s_gather = ctx.enter_context(nc.semaphore("s_gather"))
        s_act = ctx.enter_context(nc.semaphore("s_act"))
        s_dve = ctx.enter_context(nc.semaphore("s_dve"))
        block = ctx.enter_context(nc.Block())

        @block.sync
        def _(sync):
            sync.dma_start(out=gidx_sb[:, :], in_=gidx[:, :]).then_inc(s_gidx, 16)
            for g in range(NT):
                r, i = divmod(g, NTILES)
                if g >= NB:
                    # Slot reuse: wait for both compute consumers of the
                    # previous occupant.  Their completion also implies that
                    # load's completion (they waited on s_load), covering
                    # the slot WAW transitively.
                    sync.wait_ge(s_act, g - NB + 1)
                    sync.wait_ge(s_dve, g - NB + 1)
                if r >= 1:
                    # sem-reuse ordering for this tile's per-load semaphore
                    sync.wait_ge(s_load[i], 16 * r)
                sync.dma_start(
                    out=xt[g % NB][:, :], in_=x[i * P : (i + 1) * P, :]
                ).then_inc(s_load[i], 16)
            # final store after the whole epilogue
            sync.wait_ge(s_dve, NT + 8)
            sync.dma_start(out=out[:, :], in_=partial[:, :]).then_inc(s_store, 16)

        @block.gpsimd
        def _(gpsimd):
            gpsimd.wait_ge(s_gidx, 16)
            x_flat = bass.AP(tensor=x, offset=0, ap=[[1, B_LOCAL * C], [1, 1]])
            gpsimd.indirect_dma_start(
                out=xt_all[:, :],
                out_offset=None,
                in_=x_flat,
                in_offset=bass.IndirectOffsetOnAxis(ap=gidx_sb[:, :], axis=0),
            ).then_inc(s_gather, 16)

        @block.scalar
        def _(scalar):
            for g in range(NT):
                r, i = divmod(g, NTILES)
                scalar.wait_ge(s_load[i], 16 * (r + 1))
                if g >= 2:
                    # WAW on the double-buffered scratch: wait for the exp
                    # two tiles back (lagged, so the pipeline never bubbles).
                    scalar.wait_ge(s_act, g - 1)
                scalar.activation(
                    out=exp_scr[g % 2][:, :],
                    in_=xt[g % NB][:, :],
                    func=Exp,
                    accum_out=sumexp_all[:, i : i + 1],
                ).then_inc(s_act, 1)
            # epilogue: 1.01^v on the top-5, then the two logs
            scalar.wait_ge(s_dve, NT)
            scalar.activation(
                out=pw_all[:, :, :],
                in_=top8_all[:, :, 0:5],
                func=Exp,
                scale=LN101,
            ).then_inc(s_act, 1)  # -> NTILES+1
            # lse: reads this engine's own accumulator outputs; guard the
            # deep pipeline with a self-wait.
            scalar.wait_ge(s_act, NT)
            scalar.activation(
                out=lse_all[:, :], in_=sumexp_all[:, :], func=Ln
            ).then_inc(s_act, 1)  # -> NT+2
            scalar.wait_ge(s_dve, NT + 1)  # s_red ready
            scalar.activation(
                out=logs_all[:, :], in_=s_red[:, :], func=Ln
            ).then_inc(s_act, 1)  # -> NTILES+3

        @block.vector
        def _(vector):
            for g in range(NT):
                r, i = divmod(g, NTILES)
                vector.wait_ge(s_load[i], 16 * (r + 1))
                vector.max(out=top8_all[:, i, :], in_=xt[g % NB][:, :]).then_inc(
                    s_dve, 1
                )
            # epilogue
            vector.wait_ge(s_act, NT + 1)  # pw_all ready
            vector.reduce_sum(out=s_red[:, :], in_=pw_all[:, :, :], axis=X).then_inc(
                s_dve, 1
            )  # -> NTILES+1
            vector.wait_ge(s_gather, 16)
            vector.wait_ge(s_act, NT + 3)  # lse + logs ready
            # Each dependent step self-waits on the previous DVE increment:
            # the DVE pipeline gives no same-engine RAW ordering guarantee.
            # a = lse - x_t  (= -log_prob[target])
            vector.tensor_sub(
                out=a_all[:, :], in0=lse_all[:, :], in1=xt_all[:, :]
            ).then_inc(s_dve, 1)  # -> N+2
            # d = (logS - ln(1.01)*x_t) - a
            vector.scalar_tensor_tensor(
                out=d_all[:, :],
                in0=xt_all[:, :],
                scalar=-LN101,
                in1=logs_all[:, :],
                op0=Alu.mult,
                op1=Alu.add,
            ).then_inc(s_dve, 1)  # -> N+3
            vector.wait_ge(s_dve, NT + 3)
            vector.tensor_sub(
                out=d_all[:, :], in0=d_all[:, :], in1=a_all[:, :]
            ).then_inc(s_dve, 1)  # -> N+4
            # sel = x_t >= 5th-largest value
            vector.tensor_tensor(
                out=sel_all[:, :],
                in0=xt_all[:, :],
                in1=top8_all[:, :, 4],
                op=Alu.is_ge,
            ).then_inc(s_dve, 1)  # -> N+5
            vector.wait_ge(s_dve, NT + 5)
            vector.tensor_mul(
                out=d_all[:, :], in0=sel_all[:, :], in1=d_all[:, :]
            ).then_inc(s_dve, 1)  # -> N+6
            # term = 2*a + sel*d  (= (lse-x_t) - lp2)
            vector.wait_ge(s_dve, NT + 6)
            vector.scalar_tensor_tensor(
                out=term_all[:, :],
                in0=a_all[:, :],
                scalar=2.0,
                in1=d_all[:, :],
                op0=Alu.mult,
                op1=Alu.add,
            ).then_inc(s_dve, 1)  # -> N+7
            vector.wait_ge(s_dve, NT + 7)
            vector.reduce_sum(out=partial[:, :], in_=term_all[:, :], axis=X).then_inc(
                s_dve, 1
            )  # -> N+8

    return nc


def get_bass(reps=1):
    key = ("nc", reps)
    if key not in _CACHE:
        _CACHE[key] = _build_bass(reps)
    return _CACHE[key]


def make_in_maps(input, target):
    """Shard the full inputs into per-core input maps."""
    x = np.ascontiguousarray(np.asarray(input, dtype=np.float32))
    t = np.asarray(target).astype(np.int64)
    assert x.shape == (B, C), x.shape
    assert t.shape == (B,), t.shape
    rows_local = np.arange(B_LOCAL, dtype=np.int64)
    in_maps = []
    for k in range(N_CORES):
        lo = k * B_LOCAL
        flat_idx = rows_local * C + t[lo : lo + B_LOCAL]
        # gidx[p, i] = flat offset of local row i*P + p
        gidx_k = np.ascontiguousarray(
            flat_idx.reshape(NTILES, P).T.astype(np.int32)
        )
        in_maps.append({"x": x[lo : lo + B_LOCAL], "gidx": gidx_k})
    return in_maps


def reduce_outputs(results):
    """Combine per-core [P, 1] partial sums into the scalar loss."""
    total = np.float64(0.0)
    for r in results:
        total += np.asarray(r["out"], dtype=np.float64).sum()
    return np.float32(total / B)


def kernel(input, target):
    from concourse.bass_utils import run_bass_kernel_spmd

    nc = get_bass()
    in_maps = make_in_maps(input, target)
    res = run_bass_kernel_spmd(nc, in_maps, list(range(N_CORES)))
    return reduce_outputs(res.results)
